# revision 8
# baseline (speedup 1.0000x reference)
"""Trainium2 Bass kernel for diffusers AttnProcessor self-attention.

Reference computation (fp32, B=2, S=4096, C=512, H=8, D=64):
    q = hs @ Wq.T ; k = hs @ Wk.T ; v = hs @ Wv.T          (per-head split)
    probs = softmax(q k^T / sqrt(D))                        [b,h,s,s]
    out = (probs @ v) @ Wo.T + bo                           [b,s,c]

Sharding: 8 cores = (batch b in 0..1) x (query-slice of 1024 rows in 0..3).

The wall-clock of a call is dominated by the axon tunnel (~20-45 MB/s), so
the wire format is minimized:
  - each core receives ONLY its own X^T slice (xsb [C,1024] bf16, 1 MB) and
    1/8 of the packed projection weights (wsb [129,1024] bf16, 0.26 MB);
  - on device, X^T is AllGather'd within the 4-core batch group and the
    weights are AllGather'd across all 8 cores (NeuronLink, ~us);
  - the output is stored as fp16 (1 MB/core) and upcast on the host;
  - the jitted executable and the donation zero-buffers are cached across
    calls (zero h2d bytes for them after warmup).

Device dataflow per core (all matmuls bf16 in / fp32 PSUM accum):
  Xt = X[b]^T gathered in DRAM then SBUF           [C=512, S=4096]
  Qt = (Wq^T/sqrt(D)) @ Xt_q  per head-pair        [128, 1024]
  Kt = Wk^T @ Xt              per head-pair        [128, 4096]
  (a per-head copy of Qt/Kt rows is DMA'd to the opposite partition half so
   the two sq-chunks of the QK^T matmul run in disjoint PE row groups)
  V' = [X @ Wv^T | 1] per head                     [S, 65] per head
  per head h, per key tile t (128 keys):
    St[t] = Kt_h[:,t]^T Qt_h        [128 sk, 1024 sq]  (2 row-packed matmuls)
    Pt    = exp(St)                 (ScalarE, bf16 out)
    O'_h += V'[t]^T Pt              [65, 1024]  (row 64 = softmax denominator)
  O_h = O'_h[0:64] * (1/O'_h[64])   -> Ot (head-concat layout)
  out = Ot^T @ Wo^T + bo            -> DMA out  [1024, 512] fp16
"""

import numpy as np
import ml_dtypes
from contextlib import ExitStack

import jax
import concourse.bass as bass
import concourse.bacc as bacc
import concourse.mybir as mybir
import concourse.tile as tile
from concourse.bass_utils import run_bass_kernel_spmd

BF16 = mybir.dt.bfloat16
F16 = mybir.dt.float16
F32 = mybir.dt.float32
I8 = mybir.dt.int8

B, S, C, H, D = 2, 4096, 512, 8, 64
NCORES = 8
SQ = 1024          # query rows per core
P = 128            # partitions
NSK = S // P       # 32 key tiles
NCI = C // P       # 4 contraction tiles for projections
SQC = 512          # matmul moving free dim
NSQC = SQ // SQC   # 2
E = D + 1          # V' columns per head (64 v cols + ones col)

ROW_PACK = True    # run the two K=64 QK^T matmuls in disjoint PE row groups


def build_nc(row_pack=ROW_PACK, reps=1):
    nc = bacc.Bacc("TRN2", target_bir_lowering=False, debug=False,
                   num_devices=NCORES)

    # Wire inputs: this core's X^T slice (int8, absmax-scaled; the dequant
    # step is folded into Wq/Wk/Wv host-side) + 1/8 of the packed weights.
    xsb_d = nc.dram_tensor("xsb", [C, SQ], I8, kind="ExternalInput").ap()
    wsb_d = nc.dram_tensor("wsb", [P + 1, 2 * C], BF16,
                           kind="ExternalInput").ap()
    out_d = nc.dram_tensor("out", [SQ, C], F16, kind="ExternalOutput").ap()

    with ExitStack() as ctx:
        tc = ctx.enter_context(tile.TileContext(nc))
        dram = ctx.enter_context(tc.tile_pool(name="dram", bufs=1,
                                              space="DRAM"))
        const = ctx.enter_context(tc.tile_pool(name="const", bufs=1))
        work = ctx.enter_context(tc.tile_pool(name="work", bufs=2))
        psum = ctx.enter_context(tc.tile_pool(name="psum", bufs=2, space="PSUM"))

        # ---- on-device gather of X^T (per batch group) and weights (all 8)
        wbounce = dram.tile([P, 2 * C], BF16, name="wbounce", tag="wbounce")
        wall = dram.tile([4 * C, C], BF16, name="wall", tag="wall")
        xbounce = dram.tile([C, SQ], I8, name="xbounce", tag="xbounce")
        xtg = dram.tile([4 * C, SQ], I8, name="xtg", tag="xtg")

        nc.gpsimd.dma_start(wbounce[:], wsb_d[0:P, :])
        nc.gpsimd.dma_start(xbounce[:], xsb_d)
        nc.gpsimd.collective_compute(
            "AllGather", mybir.AluOpType.bypass,
            replica_groups=[list(range(NCORES))],
            ins=[wbounce.opt()], outs=[wall.opt()])
        nc.gpsimd.collective_compute(
            "AllGather", mybir.AluOpType.bypass,
            replica_groups=[[0, 1, 2, 3], [4, 5, 6, 7]],
            ins=[xbounce.opt()], outs=[xtg.opt()])

        # ---- SBUF tile loads
        # xtq straight from the wire input (no collective dependency);
        # int8 -> bf16 is exact, the dequant step lives in the weights
        xtq_sb = []
        for ci in range(NCI):
            t8 = work.tile([P, SQ], I8, name="xtq8", tag="xtq8", bufs=2)
            nc.scalar.dma_start(t8, xsb_d[ci * P:(ci + 1) * P, :])
            t = const.tile([P, SQ], BF16, name=f"xtqs{ci}", tag=f"xtqs{ci}")
            nc.vector.tensor_copy(out=t, in_=t8)
            xtq_sb.append(t)

        def load_w(base, w_i, eng):
            tiles = []
            for ci in range(NCI):
                t = const.tile([P, C], BF16, name=f"{base}{ci}",
                               tag=f"{base}{ci}")
                r0 = w_i * C + ci * P
                eng.dma_start(t, wall[r0:r0 + P, :])
                tiles.append(t)
            return tiles

        wqt_sb = load_w("wqts", 0, nc.scalar)
        wkt_sb = load_w("wkts", 1, nc.sync)
        # full X^T for K/V: xt[ci*P:(ci+1)*P, qb*SQ+sq] = xtg[qb*C+ci*P, sq]
        xt_sb = [const.tile([P, S], BF16, name=f"xts{ci}", tag=f"xts{ci}")
                 for ci in range(NCI)]
        for ck in range(S // SQC):
            qb, hf = ck // 2, ck % 2
            for ci in range(NCI):
                r0 = qb * C + ci * P
                x8 = work.tile([P, SQC], I8, name="xt8", tag="xt8", bufs=3)
                nc.sync.dma_start(
                    x8, xtg[r0:r0 + P, hf * SQC:(hf + 1) * SQC])
                nc.vector.tensor_copy(
                    out=xt_sb[ci][:, ck * SQC:(ck + 1) * SQC], in_=x8)
        wvt_sb = load_w("wvts", 2, nc.sync)
        wot_sb = load_w("wots", 3, nc.sync)

        # bo: broadcast wire row [1, C] to [P, C] fp32 via a K=1 matmul
        bo_row = const.tile([1, C], BF16, name="bo_row", tag="bo_row")
        nc.scalar.dma_start(bo_row, wsb_d[P:P + 1, 0:C])
        ones_p = const.tile([1, P], BF16, name="ones_p", tag="ones_p")
        nc.vector.memset(ones_p, 1.0)
        bob_ps = psum.tile([P, C], F32, name="bob_ps", tag="proj")
        nc.tensor.matmul(bob_ps, lhsT=ones_p, rhs=bo_row,
                         start=True, stop=True)
        bob_sb = const.tile([P, C], F32, name="bobs", tag="bobs")
        nc.vector.tensor_copy(out=bob_sb, in_=bob_ps)

        ones_sb = const.tile([P, D], F16, name="ones_sb", tag="ones_sb")
        nc.vector.memset(ones_sb, 1.0)

        for rep in range(reps):
            emit_body(nc, tc, const, work, psum, row_pack,
                      xt_sb, xtq_sb, wqt_sb, wkt_sb, wvt_sb, wot_sb,
                      bob_sb, ones_sb, out_d)

    nc.compile()
    return nc


def emit_body(nc, tc, const, work, psum, row_pack,
              xt_sb, xtq_sb, wqt_sb, wkt_sb, wvt_sb, wot_sb,
              bob_sb, ones_sb, out_d):
    vp_sb = [None] * NSK

    def emit_vproj(t_i):
        vps = psum.tile([P, C], F32, name="vps", tag="proj")
        for ci in range(NCI):
            nc.tensor.matmul(vps, lhsT=xt_sb[ci][:, t_i * P:(t_i + 1) * P],
                             rhs=wvt_sb[ci],
                             start=(ci == 0), stop=(ci == NCI - 1))
        vp = const.tile([P, H * E], BF16, name=f"vp{t_i}", tag=f"vp{t_i}")
        vp3 = vp.rearrange("p (h e) -> p h e", e=E)
        nc.vector.tensor_copy(out=vp3[:, :, 0:D],
                              in_=vps.rearrange("p (h d) -> p h d", d=D))
        nc.vector.memset(vp3[:, :, D:E], 1.0)
        vp_sb[t_i] = vp

    def emit_qtp(p):
        qtp = work.tile([P, SQ], BF16, name="qtp", tag="qtp")
        for cq in range(NSQC):
            qps = psum.tile([P, SQC], F32, name="qps", tag="proj")
            for ci in range(NCI):
                nc.tensor.matmul(
                    qps, lhsT=wqt_sb[ci][:, p * P:(p + 1) * P],
                    rhs=xtq_sb[ci][:, cq * SQC:(cq + 1) * SQC],
                    start=(ci == 0), stop=(ci == NCI - 1))
            nc.vector.tensor_copy(out=qtp[:, cq * SQC:(cq + 1) * SQC], in_=qps)
        return qtp

    def emit_ktp_chunk(ktp, p, ck):
        kps = psum.tile([P, SQC], F32, name="kps", tag="proj")
        for ci in range(NCI):
            nc.tensor.matmul(
                kps, lhsT=wkt_sb[ci][:, p * P:(p + 1) * P],
                rhs=xt_sb[ci][:, ck * SQC:(ck + 1) * SQC],
                start=(ci == 0), stop=(ci == NCI - 1))
        nc.vector.tensor_copy(out=ktp[:, ck * SQC:(ck + 1) * SQC], in_=kps)

    # Ot: normalized attention output, head-concat layout [c_in, sq]
    ot_sb = [const.tile([P, SQ], BF16, name=f"ot{i}", tag=f"ot{i}")
             for i in range(NCI)]

    def make_norm_tail(h, oraw, r):
        """Broadcast-matmul + normalize for head h. Deferred into the next
        head's loop so the PE-stream bcast matmul never waits on the DVE
        recip (PE is in-order; an early bcast would bubble the pipeline)."""
        def tail():
            rbp = psum.tile([D, SQ], F32, name="rbp", tag="st")
            for cq in range(NSQC):
                sl = slice(cq * SQC, (cq + 1) * SQC)
                nc.tensor.matmul(rbp[:, sl], lhsT=ones_sb[D:D + 1, :],
                                 rhs=r[D:D + 1, sl], start=True, stop=True)
            rb = work.tile([D, SQ], F32, name="rb", tag="rb", bufs=2)
            nc.vector.tensor_copy(out=rb, in_=rbp)
            if h % 2 == 0:
                nc.vector.tensor_mul(out=ot_sb[h // 2][0:D, :],
                                     in0=oraw[0:D, :], in1=rb)
            else:
                # DVE lanes are partition-locked; move to the upper half by DMA
                otmp = work.tile([D, SQ], BF16, name="otmp", tag="otmp",
                                 bufs=2)
                nc.vector.tensor_mul(out=otmp, in0=oraw[0:D, :], in1=rb)
                nc.gpsimd.dma_start(ot_sb[h // 2][D:2 * D, :], otmp)
        return tail

    outacc = const.tile([P, S], F16, name="outacc", tag="outacc")

    def make_oproj_tail(pair):
        """Accumulate pair `pair`'s output-projection contribution into
        outacc (SBUF). Deferred so only the final pair's slice is in the
        kernel tail."""
        def tail():
            for sqt in range(SQ // P):
                ops = psum.tile([P, C], F32, name="ops", tag="proj")
                nc.tensor.matmul(ops,
                                 lhsT=ot_sb[pair][:, sqt * P:(sqt + 1) * P],
                                 rhs=wot_sb[pair], start=True, stop=True)
                osl = outacc[:, sqt * C:(sqt + 1) * C]
                if pair == 0:
                    nc.vector.tensor_add(osl, ops, bob_sb)
                else:
                    nc.vector.tensor_add(osl, osl, ops)
            if pair == NCI - 1:
                for sqt in range(SQ // P):
                    nc.gpsimd.dma_start(
                        out_d[sqt * P:(sqt + 1) * P, :],
                        outacc[:, sqt * C:(sqt + 1) * C])
        return tail

    ktp = qtp = None
    pending_norm = None
    pending_oproj = None
    next_pair = None          # (qtp, ktp, n_chunks_pre_emitted) for pair p+1
    pre_chunks = 0
    for h in range(H):
        p, half = h // 2, h % 2
        lo, hi = half * D, half * D + D          # head's rows in pair tiles
        olo, ohi = D - half * D, 2 * D - half * D  # opposite half rows

        if half == 0:
            if next_pair is not None:
                qtp, ktp, pre_chunks = next_pair
                next_pair = None
            else:
                qtp = emit_qtp(p)
                ktp = work.tile([P, S], BF16, name="ktp", tag="ktp")
                pre_chunks = 0
        # per-head swap copies: same rows duplicated into the other
        # partition half so both sq-chunks can use disjoint PE row groups
        if row_pack:
            dma_eng = nc.gpsimd
            qts = work.tile([P, SQ], BF16, name="qts", tag="qts")
            dma_eng.dma_start(qts[olo:ohi, :], qtp[lo:hi, :])
            kts = work.tile([P, S], BF16, name="kts", tag="kts")

        def emit_k_chunk(ck):
            if half == 0 and ck >= pre_chunks:
                emit_ktp_chunk(ktp, p, ck)
            if row_pack:
                dma_eng.dma_start(
                    kts[olo:ohi, ck * SQC:(ck + 1) * SQC],
                    ktp[lo:hi, ck * SQC:(ck + 1) * SQC])

        emit_k_chunk(0)
        oacc = psum.tile([E, SQ], F32, name="oacc", tag="oacc", bufs=1)
        for t_i in range(NSK):
            # prefetch the next K chunk one window early so the QK matmuls
            # never wait on the projection->evict->swap-DMA chain
            if t_i % 4 == 1 and t_i // 4 + 1 < S // SQC:
                emit_k_chunk(t_i // 4 + 1)
            if vp_sb[t_i] is None:
                emit_vproj(t_i)
            if t_i == 8 and pending_norm is not None:
                h_prev, tail = pending_norm
                tail()
                pending_norm = None
                if h_prev % 2 == 1:
                    pending_oproj = make_oproj_tail(h_prev // 2)
            if t_i == 16 and pending_oproj is not None:
                pending_oproj()
                pending_oproj = None
            # prefetch the next pair's Q/K projections late in the second
            # head of the current pair, so the pair boundary never stalls
            # ScalarE on the projection chain
            if t_i == 24 and half == 1 and h + 1 < H and next_pair is None:
                nq = emit_qtp(p + 1)
                nk = work.tile([P, S], BF16, name="ktp", tag="ktp")
                for ck0 in range(2):
                    emit_ktp_chunk(nk, p + 1, ck0)
                next_pair = (nq, nk, 2)

            st = psum.tile([P, SQ], F32, name="st", tag="st", bufs=2)
            ksl = slice(t_i * P, (t_i + 1) * P)
            if row_pack:
                nc.tensor.matmul(st[:, 0:SQC], lhsT=ktp[lo:hi, ksl],
                                 rhs=qtp[lo:hi, 0:SQC],
                                 start=True, stop=True,
                                 tile_position=(lo, 0))
                nc.tensor.matmul(st[:, SQC:SQ], lhsT=kts[olo:ohi, ksl],
                                 rhs=qts[olo:ohi, SQC:SQ],
                                 start=True, stop=True,
                                 tile_position=(olo, 0))
            else:
                for cq in range(NSQC):
                    nc.tensor.matmul(
                        st[:, cq * SQC:(cq + 1) * SQC],
                        lhsT=ktp[lo:hi, ksl],
                        rhs=qtp[lo:hi, cq * SQC:(cq + 1) * SQC],
                        start=True, stop=True)
            pt = work.tile([P, SQ], BF16, name="pt", tag="pt", bufs=3)
            nc.scalar.activation(out=pt, in_=st,
                                 func=mybir.ActivationFunctionType.Exp)
            for cq in range(NSQC):
                nc.tensor.matmul(
                    oacc[:, cq * SQC:(cq + 1) * SQC],
                    lhsT=vp_sb[t_i][:, h * E:(h + 1) * E],
                    rhs=pt[:, cq * SQC:(cq + 1) * SQC],
                    start=(t_i == 0), stop=(t_i == NSK - 1))

        # evict oacc to SBUF immediately so the PSUM slot frees for the next
        # head; the bcast+normalize runs deferred, off the critical path
        oraw = work.tile([E, SQ], F32, name="oraw", tag="oraw", bufs=2)
        nc.vector.tensor_copy(out=oraw, in_=oacc)
        r = work.tile([E, SQ], F16, name="r", tag="r", bufs=2)
        with nc.allow_low_precision("softmax denom recip; fp16 ~1e-4 rel"):
            nc.vector.reciprocal(r[D:E, :], oraw[D:E, :])
        pending_norm = (h, make_norm_tail(h, oraw, r))

    if pending_oproj is not None:      # pair 2, if heads ended before t==16
        pending_oproj()
    pending_norm[1]()                  # final head's normalization
    make_oproj_tail(NCI - 1)()         # final pair's projection + store


def make_in_maps(hidden_states, Wq, Wk, Wv, Wo, bo):
    bf16 = ml_dtypes.bfloat16
    scale = np.float32(D) ** -0.5

    # int8 wire format for X: global absmax step, folded into Wq/Wk/Wv so
    # the device only does the (exact) int8->bf16 widening.
    hs = np.asarray(hidden_states, np.float32)
    step = np.float32(max(np.abs(hs).max() / 127.0, 1e-30))

    # packed weights: rows 0-511 Wq^T*scale*step, 512-1023 Wk^T*step,
    # 1024-1535 Wv^T*step, 1536-2047 Wo^T; viewed as [1024, 1024] for clean
    # 1/8 row shards.
    wcat = np.concatenate([
        np.asarray(Wq, np.float32).T * (scale * step),
        np.asarray(Wk, np.float32).T * step,
        np.asarray(Wv, np.float32).T * step,
        np.asarray(Wo, np.float32).T,
    ], axis=0).astype(bf16)
    wview = wcat.reshape(4 * C // 2, 2 * C)
    bo_row = np.zeros((1, 2 * C), bf16)
    bo_row[0, 0:C] = np.asarray(bo, np.float32).astype(bf16)

    # quantize in natural layout (sequential access), then transpose int8
    xq = np.clip(np.rint(hs * (np.float32(1.0) / step)), -127, 127
                 ).astype(np.int8)
    xt = [np.ascontiguousarray(xq[b].T) for b in range(B)]

    in_maps = []
    for c in range(NCORES):
        b, q0 = c // 4, (c % 4) * SQ
        wsb = np.concatenate([wview[c * P:(c + 1) * P], bo_row], axis=0)
        in_maps.append({
            "xsb": np.ascontiguousarray(xt[b][:, q0:q0 + SQ]),
            "wsb": wsb,
        })
    return in_maps


_CACHE = {}


def _get_nc():
    if "nc" not in _CACHE:
        _CACHE["nc"] = build_nc()
    return _CACHE["nc"]


def _get_exec():
    """Build (once) a cached jitted executable around the bass custom call.

    run_bass_kernel_spmd re-creates the jax.jit closure and re-uploads the
    donation zero-buffers on every call; with the axon tunnel at ~30 MB/s
    that dominates the wall clock.  Here the jit and the (never-donated,
    fully-overwritten) zero output operands live across calls.
    """
    if "exec" in _CACHE:
        return _CACHE["exec"]
    import jax.numpy  # noqa: F401  (jax initialized before first use)
    from jax.sharding import Mesh, PartitionSpec, NamedSharding
    from jax.experimental.shard_map import shard_map
    from concourse.bass2jax import (
        _bass_exec_p, install_neuronx_cc_hook, partition_id_tensor)

    nc = _get_nc()
    install_neuronx_cc_hook()
    partition_name = (nc.partition_id_tensor.name
                      if nc.partition_id_tensor else None)
    in_names, out_names, out_avals, zero_outs = [], [], [], []
    for alloc in nc.m.functions[0].allocations:
        if not isinstance(alloc, mybir.MemoryLocationSet):
            continue
        name = alloc.memorylocations[0].name
        if alloc.kind == "ExternalInput":
            if name != partition_name:
                in_names.append(name)
        elif alloc.kind == "ExternalOutput":
            shape = tuple(alloc.tensor_shape)
            dtype = mybir.dt.np(alloc.dtype)
            out_names.append(name)
            out_avals.append(jax.core.ShapedArray(shape, dtype))
            zero_outs.append(np.zeros(shape, dtype))
    n_params = len(in_names)
    in_names_all = list(in_names) + out_names
    if partition_name is not None:
        in_names_all.append(partition_name)

    def _body(*args):
        operands = list(args)
        if partition_name is not None:
            operands.append(partition_id_tensor())
        outs = _bass_exec_p.bind(
            *operands,
            out_avals=tuple(out_avals),
            in_names=tuple(in_names_all),
            out_names=tuple(out_names),
            lowering_input_output_aliases=(),
            sim_require_finite=True,
            sim_require_nnan=True,
            nc=nc,
        )
        return tuple(outs)

    devices = jax.devices()[:NCORES]
    mesh = Mesh(np.asarray(devices), ("core",))
    n_outs = len(out_avals)
    sharded = jax.jit(
        shard_map(_body, mesh=mesh,
                  in_specs=(PartitionSpec("core"),) * (n_params + n_outs),
                  out_specs=(PartitionSpec("core"),) * n_outs,
                  check_rep=False),
        keep_unused=True,
    )
    sh = NamedSharding(mesh, PartitionSpec("core"))
    # zero output operands: device-resident, reused every call (not donated)
    dev_zeros = [
        jax.device_put(
            np.zeros((NCORES * z.shape[0], *z.shape[1:]), z.dtype), sh)
        for z in zero_outs
    ]
    _CACHE["exec"] = (sharded, in_names, out_names, out_avals, dev_zeros)
    return _CACHE["exec"]


def run(inputs, trace=False, **kwargs):
    """Run on hardware; returns (full_output [B,S,C] fp32, results)."""
    if trace:
        nc = _get_nc()
        in_maps = make_in_maps(**inputs)
        res = run_bass_kernel_spmd(nc, in_maps, list(range(NCORES)),
                                   trace=True, **kwargs)
        out = np.empty((B, S, C), np.float32)
        for c in range(NCORES):
            b, q0 = c // 4, (c % 4) * SQ
            out[b, q0:q0 + SQ, :] = np.asarray(
                res.results[c]["out"], np.float32)
        return out, res

    sharded, in_names, out_names, out_avals, dev_zeros = _get_exec()
    in_maps = make_in_maps(**inputs)
    concat_in = [
        np.concatenate([in_maps[c][name] for c in range(NCORES)], axis=0)
        for name in in_names
    ]
    out_arrs = sharded(*concat_in, *dev_zeros)
    out = np.empty((B, S, C), np.float32)
    per_core = np.asarray(out_arrs[out_names.index("out")])
    per_core = per_core.reshape(NCORES, SQ, C)
    for c in range(NCORES):
        b, q0 = c // 4, (c % 4) * SQ
        out[b, q0:q0 + SQ, :] = per_core[c].astype(np.float32)
    return out, None


def kernel(**inputs):
    out, _ = run(inputs)
    return out


# revision 17
# speedup vs baseline: 1.0656x; 1.0656x over previous
"""Trainium2 Bass kernel for diffusers AttnProcessor self-attention.

Reference computation (fp32, B=2, S=4096, C=512, H=8, D=64):
    q = hs @ Wq.T ; k = hs @ Wk.T ; v = hs @ Wv.T          (per-head split)
    probs = softmax(q k^T / sqrt(D))                        [b,h,s,s]
    out = (probs @ v) @ Wo.T + bo                           [b,s,c]

Sharding: 8 cores = (batch b in 0..1) x (query-slice of 1024 rows in 0..3).

The wall-clock of a call is dominated by the axon tunnel (~20-45 MB/s), so
the wire format is minimized:
  - each core receives ONLY its own X^T slice (xsb [C,1024] bf16, 1 MB) and
    1/8 of the packed projection weights (wsb [129,1024] bf16, 0.26 MB);
  - on device, X^T is AllGather'd within the 4-core batch group and the
    weights are AllGather'd across all 8 cores (NeuronLink, ~us);
  - the output is stored as fp16 (1 MB/core) and upcast on the host;
  - the jitted executable and the donation zero-buffers are cached across
    calls (zero h2d bytes for them after warmup).

Device dataflow per core (all matmuls bf16 in / fp32 PSUM accum):
  Xt = X[b]^T gathered in DRAM then SBUF           [C=512, S=4096]
  Qt = (Wq^T/sqrt(D)) @ Xt_q  per head-pair        [128, 1024]
  Kt = Wk^T @ Xt              per head-pair        [128, 4096]
  (a per-head copy of Qt/Kt rows is DMA'd to the opposite partition half so
   the two sq-chunks of the QK^T matmul run in disjoint PE row groups)
  V' = [X @ Wv^T | 1] per head                     [S, 65] per head
  per head h, per key tile t (128 keys):
    St[t] = Kt_h[:,t]^T Qt_h        [128 sk, 1024 sq]  (2 row-packed matmuls)
    Pt    = exp(St)                 (ScalarE, bf16 out)
    O'_h += V'[t]^T Pt              [65, 1024]  (row 64 = softmax denominator)
  O_h = O'_h[0:64] * (1/O'_h[64])   -> Ot (head-concat layout)
  out = Ot^T @ Wo^T + bo            -> DMA out  [1024, 512] fp16
"""

import numpy as np
import ml_dtypes
from contextlib import ExitStack

import jax
import concourse.bass as bass
import concourse.bacc as bacc
import concourse.mybir as mybir
import concourse.tile as tile
from concourse.bass_utils import run_bass_kernel_spmd

BF16 = mybir.dt.bfloat16
F16 = mybir.dt.float16
F32 = mybir.dt.float32
I8 = mybir.dt.int8

B, S, C, H, D = 2, 4096, 512, 8, 64
NCORES = 8
SQ = 1024          # query rows per core
P = 128            # partitions
NSK = S // P       # 32 key tiles
NCI = C // P       # 4 contraction tiles for projections
SQC = 512          # matmul moving free dim
NSQC = SQ // SQC   # 2
E = D + 1          # V' columns per head (64 v cols + ones col)

ROW_PACK = True    # run the two K=64 QK^T matmuls in disjoint PE row groups


def build_nc(row_pack=ROW_PACK, reps=1):
    nc = bacc.Bacc("TRN2", target_bir_lowering=False, debug=False,
                   num_devices=NCORES)

    # Single wire input per core (one RPC per device over the axon tunnel):
    # rows 0-511   X^T slice, int8 (absmax step folded into Wq/Wk/Wv)
    # rows 512-767 this core's 1/8 of the packed bf16 weights (bitcast view)
    # row  768     bo (bf16, replicated)
    blob_d = nc.dram_tensor("blob", [C + 2 * P + 1, SQ], I8,
                            kind="ExternalInput").ap()
    # Output gathered on-device across all 8 cores -> replicated, so the
    # host fetches the full result from one device in one RPC.
    out_d = nc.dram_tensor("out", [NCORES * SQ, C], F16,
                           kind="ExternalOutput").ap()
    xsb_d = blob_d[0:C, :]
    w_ap = blob_d[C:C + 2 * P, :].bitcast(BF16)          # [256, 512]
    bo_ap = blob_d[C + 2 * P:C + 2 * P + 1, :].bitcast(BF16)  # [1, 512]

    with ExitStack() as ctx:
        tc = ctx.enter_context(tile.TileContext(nc))
        dram = ctx.enter_context(tc.tile_pool(name="dram", bufs=1,
                                              space="DRAM"))
        const = ctx.enter_context(tc.tile_pool(name="const", bufs=1))
        work = ctx.enter_context(tc.tile_pool(name="work", bufs=2))
        psum = ctx.enter_context(tc.tile_pool(name="psum", bufs=2, space="PSUM"))

        # ---- on-device gather of X^T (per batch group) and weights (all 8)
        wbounce = dram.tile([2 * P, C], BF16, name="wbounce", tag="wbounce")
        wall = dram.tile([4 * C, C], BF16, name="wall", tag="wall")
        xbounce = dram.tile([C, SQ], I8, name="xbounce", tag="xbounce")
        xtg = dram.tile([4 * C, SQ], I8, name="xtg", tag="xtg")

        nc.gpsimd.dma_start(wbounce[:], w_ap)
        nc.gpsimd.dma_start(xbounce[:], xsb_d)
        nc.gpsimd.collective_compute(
            "AllGather", mybir.AluOpType.bypass,
            replica_groups=[list(range(NCORES))],
            ins=[wbounce.opt()], outs=[wall.opt()])
        nc.gpsimd.collective_compute(
            "AllGather", mybir.AluOpType.bypass,
            replica_groups=[[0, 1, 2, 3], [4, 5, 6, 7]],
            ins=[xbounce.opt()], outs=[xtg.opt()])

        # ---- SBUF tile loads
        # xtq straight from the wire input (no collective dependency);
        # int8 -> bf16 is exact, the dequant step lives in the weights
        xtq_sb = []
        for ci in range(NCI):
            t8 = work.tile([P, SQ], I8, name="xtq8", tag="xtq8", bufs=2)
            nc.scalar.dma_start(t8, xsb_d[ci * P:(ci + 1) * P, :])
            t = const.tile([P, SQ], BF16, name=f"xtqs{ci}", tag=f"xtqs{ci}")
            nc.vector.tensor_copy(out=t, in_=t8)
            xtq_sb.append(t)

        def load_w(base, w_i, eng):
            tiles = []
            for ci in range(NCI):
                t = const.tile([P, C], BF16, name=f"{base}{ci}",
                               tag=f"{base}{ci}")
                r0 = w_i * C + ci * P
                eng.dma_start(t, wall[r0:r0 + P, :])
                tiles.append(t)
            return tiles

        wqt_sb = load_w("wqts", 0, nc.scalar)
        wkt_sb = load_w("wkts", 1, nc.sync)
        # full X^T for K/V: xt[ci*P:(ci+1)*P, qb*SQ+sq] = xtg[qb*C+ci*P, sq]
        xt_sb = [const.tile([P, S], BF16, name=f"xts{ci}", tag=f"xts{ci}")
                 for ci in range(NCI)]
        for ck in range(S // SQC):
            qb, hf = ck // 2, ck % 2
            for ci in range(NCI):
                r0 = qb * C + ci * P
                x8 = work.tile([P, SQC], I8, name="xt8", tag="xt8", bufs=3)
                nc.sync.dma_start(
                    x8, xtg[r0:r0 + P, hf * SQC:(hf + 1) * SQC])
                nc.vector.tensor_copy(
                    out=xt_sb[ci][:, ck * SQC:(ck + 1) * SQC], in_=x8)
        wvt_sb = load_w("wvts", 2, nc.sync)
        wot_sb = load_w("wots", 3, nc.sync)

        # bo: broadcast wire row [1, C] to [P, C] fp32 via a K=1 matmul
        bo_row = const.tile([1, C], BF16, name="bo_row", tag="bo_row")
        nc.scalar.dma_start(bo_row, bo_ap)
        ones_p = const.tile([1, P], BF16, name="ones_p", tag="ones_p")
        nc.vector.memset(ones_p, 1.0)
        bob_ps = psum.tile([P, C], F32, name="bob_ps", tag="proj")
        nc.tensor.matmul(bob_ps, lhsT=ones_p, rhs=bo_row,
                         start=True, stop=True)
        bob_sb = const.tile([P, C], F32, name="bobs", tag="bobs")
        nc.vector.tensor_copy(out=bob_sb, in_=bob_ps)

        ones_sb = const.tile([P, D], F16, name="ones_sb", tag="ones_sb")
        nc.vector.memset(ones_sb, 1.0)

        obounce = dram.tile([SQ, C], F16, name="obounce", tag="obounce")
        og = dram.tile([NCORES * SQ, C], F16, name="og", tag="og")
        for rep in range(reps):
            emit_body(nc, tc, const, work, psum, row_pack,
                      xt_sb, xtq_sb, wqt_sb, wkt_sb, wvt_sb, wot_sb,
                      bob_sb, ones_sb, obounce)
        nc.gpsimd.collective_compute(
            "AllGather", mybir.AluOpType.bypass,
            replica_groups=[list(range(NCORES))],
            ins=[obounce.opt()], outs=[og.opt()])
        nc.gpsimd.dma_start(out_d, og[:])

    nc.compile()
    return nc


def emit_body(nc, tc, const, work, psum, row_pack,
              xt_sb, xtq_sb, wqt_sb, wkt_sb, wvt_sb, wot_sb,
              bob_sb, ones_sb, out_d):
    vp_sb = [None] * NSK

    def emit_vproj(t_i):
        vps = psum.tile([P, C], F32, name="vps", tag="proj")
        for ci in range(NCI):
            nc.tensor.matmul(vps, lhsT=xt_sb[ci][:, t_i * P:(t_i + 1) * P],
                             rhs=wvt_sb[ci],
                             start=(ci == 0), stop=(ci == NCI - 1))
        vp = const.tile([P, H * E], BF16, name=f"vp{t_i}", tag=f"vp{t_i}")
        vp3 = vp.rearrange("p (h e) -> p h e", e=E)
        nc.vector.tensor_copy(out=vp3[:, :, 0:D],
                              in_=vps.rearrange("p (h d) -> p h d", d=D))
        nc.vector.memset(vp3[:, :, D:E], 1.0)
        vp_sb[t_i] = vp

    def emit_qtp(p):
        qtp = work.tile([P, SQ], BF16, name="qtp", tag="qtp")
        for cq in range(NSQC):
            qps = psum.tile([P, SQC], F32, name="qps", tag="proj")
            for ci in range(NCI):
                nc.tensor.matmul(
                    qps, lhsT=wqt_sb[ci][:, p * P:(p + 1) * P],
                    rhs=xtq_sb[ci][:, cq * SQC:(cq + 1) * SQC],
                    start=(ci == 0), stop=(ci == NCI - 1))
            nc.vector.tensor_copy(out=qtp[:, cq * SQC:(cq + 1) * SQC], in_=qps)
        return qtp

    def emit_ktp_chunk(ktp, p, ck):
        kps = psum.tile([P, SQC], F32, name="kps", tag="proj")
        for ci in range(NCI):
            nc.tensor.matmul(
                kps, lhsT=wkt_sb[ci][:, p * P:(p + 1) * P],
                rhs=xt_sb[ci][:, ck * SQC:(ck + 1) * SQC],
                start=(ci == 0), stop=(ci == NCI - 1))
        nc.vector.tensor_copy(out=ktp[:, ck * SQC:(ck + 1) * SQC], in_=kps)

    # Ot: normalized attention output, head-concat layout [c_in, sq]
    ot_sb = [const.tile([P, SQ], BF16, name=f"ot{i}", tag=f"ot{i}")
             for i in range(NCI)]

    def make_norm_tail(h, oraw, r):
        """Broadcast-matmul + normalize for head h. Deferred into the next
        head's loop so the PE-stream bcast matmul never waits on the DVE
        recip (PE is in-order; an early bcast would bubble the pipeline)."""
        def tail():
            rbp = psum.tile([D, SQ], F32, name="rbp", tag="st")
            for cq in range(NSQC):
                sl = slice(cq * SQC, (cq + 1) * SQC)
                nc.tensor.matmul(rbp[:, sl], lhsT=ones_sb[D:D + 1, :],
                                 rhs=r[D:D + 1, sl], start=True, stop=True)
            rb = work.tile([D, SQ], F32, name="rb", tag="rb", bufs=2)
            nc.vector.tensor_copy(out=rb, in_=rbp)
            if h % 2 == 0:
                nc.vector.tensor_mul(out=ot_sb[h // 2][0:D, :],
                                     in0=oraw[0:D, :], in1=rb)
            else:
                # DVE lanes are partition-locked; move to the upper half by DMA
                otmp = work.tile([D, SQ], BF16, name="otmp", tag="otmp",
                                 bufs=2)
                nc.vector.tensor_mul(out=otmp, in0=oraw[0:D, :], in1=rb)
                nc.gpsimd.dma_start(ot_sb[h // 2][D:2 * D, :], otmp)
        return tail

    outacc = const.tile([P, S], F16, name="outacc", tag="outacc")

    def make_oproj_tail(pair):
        """Accumulate pair `pair`'s output-projection contribution into
        outacc (SBUF). Deferred so only the final pair's slice is in the
        kernel tail."""
        def tail():
            for sqt in range(SQ // P):
                ops = psum.tile([P, C], F32, name="ops", tag="proj")
                nc.tensor.matmul(ops,
                                 lhsT=ot_sb[pair][:, sqt * P:(sqt + 1) * P],
                                 rhs=wot_sb[pair], start=True, stop=True)
                osl = outacc[:, sqt * C:(sqt + 1) * C]
                if pair == 0:
                    nc.vector.tensor_add(osl, ops, bob_sb)
                else:
                    nc.vector.tensor_add(osl, osl, ops)
            if pair == NCI - 1:
                for sqt in range(SQ // P):
                    nc.gpsimd.dma_start(
                        out_d[sqt * P:(sqt + 1) * P, :],
                        outacc[:, sqt * C:(sqt + 1) * C])
        return tail

    ktp = qtp = None
    pending_norm = None
    pending_oproj = None
    next_pair = None          # (qtp, ktp, n_chunks_pre_emitted) for pair p+1
    pre_chunks = 0
    for h in range(H):
        p, half = h // 2, h % 2
        lo, hi = half * D, half * D + D          # head's rows in pair tiles
        olo, ohi = D - half * D, 2 * D - half * D  # opposite half rows

        if half == 0:
            if next_pair is not None:
                qtp, ktp, pre_chunks = next_pair
                next_pair = None
            else:
                qtp = emit_qtp(p)
                ktp = work.tile([P, S], BF16, name="ktp", tag="ktp")
                pre_chunks = 0
        # per-head swap copies: same rows duplicated into the other
        # partition half so both sq-chunks can use disjoint PE row groups
        if row_pack:
            dma_eng = nc.gpsimd
            qts = work.tile([P, SQ], BF16, name="qts", tag="qts")
            dma_eng.dma_start(qts[olo:ohi, :], qtp[lo:hi, :])
            kts = work.tile([P, S], BF16, name="kts", tag="kts")

        def emit_k_chunk(ck):
            if half == 0 and ck >= pre_chunks:
                emit_ktp_chunk(ktp, p, ck)
            if row_pack:
                dma_eng.dma_start(
                    kts[olo:ohi, ck * SQC:(ck + 1) * SQC],
                    ktp[lo:hi, ck * SQC:(ck + 1) * SQC])

        emit_k_chunk(0)
        oacc = psum.tile([E, SQ], F32, name="oacc", tag="oacc", bufs=1)
        for t_i in range(NSK):
            # prefetch the next K chunk one window early so the QK matmuls
            # never wait on the projection->evict->swap-DMA chain
            if t_i % 4 == 1 and t_i // 4 + 1 < S // SQC:
                emit_k_chunk(t_i // 4 + 1)
            if vp_sb[t_i] is None:
                emit_vproj(t_i)
            if t_i == 8 and pending_norm is not None:
                h_prev, tail = pending_norm
                tail()
                pending_norm = None
                if h_prev % 2 == 1:
                    pending_oproj = make_oproj_tail(h_prev // 2)
            if t_i == 16 and pending_oproj is not None:
                pending_oproj()
                pending_oproj = None
            # prefetch the next pair's Q/K projections late in the second
            # head of the current pair, so the pair boundary never stalls
            # ScalarE on the projection chain
            if t_i == 24 and half == 1 and h + 1 < H and next_pair is None:
                nq = emit_qtp(p + 1)
                nk = work.tile([P, S], BF16, name="ktp", tag="ktp")
                for ck0 in range(2):
                    emit_ktp_chunk(nk, p + 1, ck0)
                next_pair = (nq, nk, 2)

            st = psum.tile([P, SQ], F32, name="st", tag="st", bufs=2)
            ksl = slice(t_i * P, (t_i + 1) * P)
            if row_pack:
                nc.tensor.matmul(st[:, 0:SQC], lhsT=ktp[lo:hi, ksl],
                                 rhs=qtp[lo:hi, 0:SQC],
                                 start=True, stop=True,
                                 tile_position=(lo, 0))
                nc.tensor.matmul(st[:, SQC:SQ], lhsT=kts[olo:ohi, ksl],
                                 rhs=qts[olo:ohi, SQC:SQ],
                                 start=True, stop=True,
                                 tile_position=(olo, 0))
            else:
                for cq in range(NSQC):
                    nc.tensor.matmul(
                        st[:, cq * SQC:(cq + 1) * SQC],
                        lhsT=ktp[lo:hi, ksl],
                        rhs=qtp[lo:hi, cq * SQC:(cq + 1) * SQC],
                        start=True, stop=True)
            pt = work.tile([P, SQ], BF16, name="pt", tag="pt", bufs=3)
            nc.scalar.activation(out=pt, in_=st,
                                 func=mybir.ActivationFunctionType.Exp)
            for cq in range(NSQC):
                nc.tensor.matmul(
                    oacc[:, cq * SQC:(cq + 1) * SQC],
                    lhsT=vp_sb[t_i][:, h * E:(h + 1) * E],
                    rhs=pt[:, cq * SQC:(cq + 1) * SQC],
                    start=(t_i == 0), stop=(t_i == NSK - 1))

        # evict oacc to SBUF immediately so the PSUM slot frees for the next
        # head; the bcast+normalize runs deferred, off the critical path
        oraw = work.tile([E, SQ], F32, name="oraw", tag="oraw", bufs=2)
        nc.vector.tensor_copy(out=oraw, in_=oacc)
        r = work.tile([E, SQ], F16, name="r", tag="r", bufs=2)
        with nc.allow_low_precision("softmax denom recip; fp16 ~1e-4 rel"):
            nc.vector.reciprocal(r[D:E, :], oraw[D:E, :])
        pending_norm = (h, make_norm_tail(h, oraw, r))

    if pending_oproj is not None:      # pair 2, if heads ended before t==16
        pending_oproj()
    pending_norm[1]()                  # final head's normalization
    make_oproj_tail(NCI - 1)()         # final pair's projection + store


def make_in_maps(hidden_states, Wq, Wk, Wv, Wo, bo):
    bf16 = ml_dtypes.bfloat16
    scale = np.float32(D) ** -0.5

    # int8 wire format for X: global absmax step, folded into Wq/Wk/Wv so
    # the device only does the (exact) int8->bf16 widening.
    hs = np.asarray(hidden_states, np.float32)
    step = np.float32(max(np.abs(hs).max() / 127.0, 1e-30))

    # packed weights: rows 0-511 Wq^T*scale*step, 512-1023 Wk^T*step,
    # 1024-1535 Wv^T*step, 1536-2047 Wo^T; 1/8 row shard per core.
    wcat = np.concatenate([
        np.asarray(Wq, np.float32).T * (scale * step),
        np.asarray(Wk, np.float32).T * step,
        np.asarray(Wv, np.float32).T * step,
        np.asarray(Wo, np.float32).T,
    ], axis=0).astype(bf16, order="C")
    wbytes = wcat.view(np.int8).reshape(NCORES, 2 * P, SQ)
    bo_row = np.zeros((1, SQ), np.int8)
    bo_row[0:1, 0:2 * C] = np.asarray(bo, np.float32).astype(bf16).reshape(
        1, C).view(np.int8)

    # quantize in natural layout (sequential access), then transpose int8
    xq = np.clip(np.rint(hs * (np.float32(1.0) / step)), -127, 127
                 ).astype(np.int8)
    xt = [np.ascontiguousarray(xq[b].T) for b in range(B)]

    in_maps = []
    for c in range(NCORES):
        b, q0 = c // 4, (c % 4) * SQ
        blob = np.concatenate(
            [xt[b][:, q0:q0 + SQ], wbytes[c], bo_row], axis=0)
        in_maps.append({"blob": blob})
    return in_maps


_CACHE = {}


def _get_nc():
    if "nc" not in _CACHE:
        _CACHE["nc"] = build_nc()
    return _CACHE["nc"]


def _get_exec():
    """Build (once) a cached jitted executable around the bass custom call.

    run_bass_kernel_spmd re-creates the jax.jit closure and re-uploads the
    donation zero-buffers on every call; with the axon tunnel at ~30 MB/s
    that dominates the wall clock.  Here the jit and the (never-donated,
    fully-overwritten) zero output operands live across calls.
    """
    if "exec" in _CACHE:
        return _CACHE["exec"]
    import jax.numpy  # noqa: F401  (jax initialized before first use)
    from jax.sharding import Mesh, PartitionSpec, NamedSharding
    from jax.experimental.shard_map import shard_map
    from concourse.bass2jax import (
        _bass_exec_p, install_neuronx_cc_hook, partition_id_tensor)

    nc = _get_nc()
    install_neuronx_cc_hook()
    partition_name = (nc.partition_id_tensor.name
                      if nc.partition_id_tensor else None)
    in_names, out_names, out_avals, zero_outs = [], [], [], []
    for alloc in nc.m.functions[0].allocations:
        if not isinstance(alloc, mybir.MemoryLocationSet):
            continue
        name = alloc.memorylocations[0].name
        if alloc.kind == "ExternalInput":
            if name != partition_name:
                in_names.append(name)
        elif alloc.kind == "ExternalOutput":
            shape = tuple(alloc.tensor_shape)
            dtype = mybir.dt.np(alloc.dtype)
            out_names.append(name)
            out_avals.append(jax.core.ShapedArray(shape, dtype))
            zero_outs.append(np.zeros(shape, dtype))
    n_params = len(in_names)
    in_names_all = list(in_names) + out_names
    if partition_name is not None:
        in_names_all.append(partition_name)

    def _body(*args):
        operands = list(args)
        if partition_name is not None:
            operands.append(partition_id_tensor())
        outs = _bass_exec_p.bind(
            *operands,
            out_avals=tuple(out_avals),
            in_names=tuple(in_names_all),
            out_names=tuple(out_names),
            lowering_input_output_aliases=(),
            sim_require_finite=True,
            sim_require_nnan=True,
            nc=nc,
        )
        return tuple(outs)

    devices = jax.devices()[:NCORES]
    mesh = Mesh(np.asarray(devices), ("core",))
    n_outs = len(out_avals)
    # outputs are gathered on-device across all 8 cores -> replicated specs,
    # so the host fetches the full result from one device in one RPC
    sharded = jax.jit(
        shard_map(_body, mesh=mesh,
                  in_specs=(PartitionSpec("core"),) * n_params
                  + (PartitionSpec(),) * n_outs,
                  out_specs=(PartitionSpec(),) * n_outs,
                  check_rep=False),
        keep_unused=True,
    )
    # zero output operands: created ON DEVICE (no tunnel transfer),
    # device-resident and reused every call (not donated)
    import jax.numpy as jnp
    mk_zeros = jax.jit(
        lambda: tuple(jnp.zeros(a.shape, a.dtype) for a in out_avals),
        out_shardings=NamedSharding(mesh, PartitionSpec()),
    )
    dev_zeros = list(mk_zeros())
    _CACHE["exec"] = (sharded, in_names, out_names, out_avals, dev_zeros)
    return _CACHE["exec"]


def run(inputs, trace=False, **kwargs):
    """Run on hardware; returns (full_output [B,S,C] fp32, results)."""
    if trace:
        nc = _get_nc()
        in_maps = make_in_maps(**inputs)
        res = run_bass_kernel_spmd(nc, in_maps, list(range(NCORES)),
                                   trace=True, **kwargs)
        og = np.asarray(res.results[0]["out"])
    else:
        sharded, in_names, out_names, out_avals, dev_zeros = _get_exec()
        in_maps = make_in_maps(**inputs)
        concat_in = [
            np.concatenate([in_maps[c][name] for c in range(NCORES)], axis=0)
            for name in in_names
        ]
        out_arrs = sharded(*concat_in, *dev_zeros)
        res = None
        og = np.asarray(out_arrs[out_names.index("out")])

    og = og.reshape(NCORES, SQ, C)
    out = np.empty((B, S, C), np.float32)
    for c in range(NCORES):
        b, q0 = c // 4, (c % 4) * SQ
        out[b, q0:q0 + SQ, :] = og[c].astype(np.float32)
    return out, res


def kernel(**inputs):
    try:
        out, _ = run(inputs)
    except Exception:
        # The axon fleet occasionally reports NRT_EXEC_UNIT_UNRECOVERABLE
        # once after a prior session's comm state; rebuild and retry once.
        _CACHE.clear()
        out, _ = run(inputs)
    return out


# revision 24
# speedup vs baseline: 1.9513x; 1.8311x over previous
"""Trainium2 Bass kernel for diffusers AttnProcessor self-attention.

Reference computation (fp32, B=2, S=4096, C=512, H=8, D=64):
    q = hs @ Wq.T ; k = hs @ Wk.T ; v = hs @ Wv.T          (per-head split)
    probs = softmax(q k^T / sqrt(D))                        [b,h,s,s]
    out = (probs @ v) @ Wo.T + bo                           [b,s,c]

Sharding: 8 cores = (batch b in 0..1) x (query-slice of 1024 rows in 0..3).

The wall-clock of a call is dominated by the axon tunnel (~20-45 MB/s), so
the wire format is minimized:
  - each core receives ONLY its own X^T slice (xsb [C,1024] bf16, 1 MB) and
    1/8 of the packed projection weights (wsb [129,1024] bf16, 0.26 MB);
  - on device, X^T is AllGather'd within the 4-core batch group and the
    weights are AllGather'd across all 8 cores (NeuronLink, ~us);
  - the output is stored as fp16 (1 MB/core) and upcast on the host;
  - the jitted executable and the donation zero-buffers are cached across
    calls (zero h2d bytes for them after warmup).

Device dataflow per core (all matmuls bf16 in / fp32 PSUM accum):
  Xt = X[b]^T gathered in DRAM then SBUF           [C=512, S=4096]
  Qt = (Wq^T/sqrt(D)) @ Xt_q  per head-pair        [128, 1024]
  Kt = Wk^T @ Xt              per head-pair        [128, 4096]
  (a per-head copy of Qt/Kt rows is DMA'd to the opposite partition half so
   the two sq-chunks of the QK^T matmul run in disjoint PE row groups)
  V' = [X @ Wv^T | 1] per head                     [S, 65] per head
  per head h, per key tile t (128 keys):
    St[t] = Kt_h[:,t]^T Qt_h        [128 sk, 1024 sq]  (2 row-packed matmuls)
    Pt    = exp(St)                 (ScalarE, bf16 out)
    O'_h += V'[t]^T Pt              [65, 1024]  (row 64 = softmax denominator)
  O_h = O'_h[0:64] * (1/O'_h[64])   -> Ot (head-concat layout)
  out = Ot^T @ Wo^T + bo            -> DMA out  [1024, 512] fp16
"""

import numpy as np
import ml_dtypes
from contextlib import ExitStack

import jax
import concourse.bass as bass
import concourse.bacc as bacc
import concourse.mybir as mybir
import concourse.tile as tile
from concourse.bass_utils import run_bass_kernel_spmd

BF16 = mybir.dt.bfloat16
F16 = mybir.dt.float16
F32 = mybir.dt.float32
I8 = mybir.dt.int8

B, S, C, H, D = 2, 4096, 512, 8, 64
NCORES = 8
SQ = 1024          # query rows per core
P = 128            # partitions
NSK = S // P       # 32 key tiles
NCI = C // P       # 4 contraction tiles for projections
SQC = 512          # matmul moving free dim
NSQC = SQ // SQC   # 2
E = D + 1          # V' columns per head (64 v cols + ones col)

ROW_PACK = True    # run the two K=64 QK^T matmuls in disjoint PE row groups


def build_nc(row_pack=ROW_PACK, reps=1):
    nc = bacc.Bacc("TRN2", target_bir_lowering=False, debug=False,
                   num_devices=NCORES)

    # Wire inputs, split so the weights can stay device-resident between
    # calls (re-uploaded only when the weight/step hash changes):
    #   xblob: this core's X^T slice, int8 (absmax step folded into Wq/Wk/Wv)
    #   wblob: rows 0-255 this core's 1/8 of the packed bf16 weights,
    #          row 256 bo (bf16, replicated)
    xsb_d = nc.dram_tensor("xblob", [C, SQ], I8, kind="ExternalInput").ap()
    wblob_d = nc.dram_tensor("wblob", [2 * P + 1, C], BF16,
                             kind="ExternalInput").ap()
    # Output: per-token int8 + 8 rows of bitcast fp32 row-maxima, gathered
    # on-device across all 8 cores -> replicated, so the host fetches the
    # full result from one device in one RPC.
    out_d = nc.dram_tensor("out", [NCORES * (SQ + 8), C], I8,
                           kind="ExternalOutput").ap()
    w_ap = wblob_d[0:2 * P, :]
    bo_ap = wblob_d[2 * P:2 * P + 1, :]

    with ExitStack() as ctx:
        tc = ctx.enter_context(tile.TileContext(nc))
        dram = ctx.enter_context(tc.tile_pool(name="dram", bufs=1,
                                              space="DRAM"))
        const = ctx.enter_context(tc.tile_pool(name="const", bufs=1))
        work = ctx.enter_context(tc.tile_pool(name="work", bufs=2))
        psum = ctx.enter_context(tc.tile_pool(name="psum", bufs=2, space="PSUM"))

        # ---- on-device gather of X^T (per batch group) and weights (all 8)
        wbounce = dram.tile([2 * P, C], BF16, name="wbounce", tag="wbounce")
        wall = dram.tile([4 * C, C], BF16, name="wall", tag="wall")
        xbounce = dram.tile([C, SQ], I8, name="xbounce", tag="xbounce")
        xtg = dram.tile([4 * C, SQ], I8, name="xtg", tag="xtg")

        nc.gpsimd.dma_start(wbounce[:], w_ap)
        nc.gpsimd.dma_start(xbounce[:], xsb_d)
        nc.gpsimd.collective_compute(
            "AllGather", mybir.AluOpType.bypass,
            replica_groups=[list(range(NCORES))],
            ins=[wbounce.opt()], outs=[wall.opt()])
        nc.gpsimd.collective_compute(
            "AllGather", mybir.AluOpType.bypass,
            replica_groups=[[0, 1, 2, 3], [4, 5, 6, 7]],
            ins=[xbounce.opt()], outs=[xtg.opt()])

        # ---- SBUF tile loads
        # xtq straight from the wire input (no collective dependency);
        # int8 -> bf16 is exact, the dequant step lives in the weights
        xtq_sb = []
        for ci in range(NCI):
            t8 = work.tile([P, SQ], I8, name="xtq8", tag="xtq8", bufs=2)
            nc.scalar.dma_start(t8, xsb_d[ci * P:(ci + 1) * P, :])
            t = const.tile([P, SQ], BF16, name=f"xtqs{ci}", tag=f"xtqs{ci}")
            nc.vector.tensor_copy(out=t, in_=t8)
            xtq_sb.append(t)

        def load_w(base, w_i, eng):
            tiles = []
            for ci in range(NCI):
                t = const.tile([P, C], BF16, name=f"{base}{ci}",
                               tag=f"{base}{ci}")
                r0 = w_i * C + ci * P
                eng.dma_start(t, wall[r0:r0 + P, :])
                tiles.append(t)
            return tiles

        wqt_sb = load_w("wqts", 0, nc.scalar)
        wkt_sb = load_w("wkts", 1, nc.sync)
        # full X^T for K/V: xt[ci*P:(ci+1)*P, qb*SQ+sq] = xtg[qb*C+ci*P, sq]
        xt_sb = [const.tile([P, S], BF16, name=f"xts{ci}", tag=f"xts{ci}")
                 for ci in range(NCI)]
        for ck in range(S // SQC):
            qb, hf = ck // 2, ck % 2
            for ci in range(NCI):
                r0 = qb * C + ci * P
                x8 = work.tile([P, SQC], I8, name="xt8", tag="xt8", bufs=3)
                nc.sync.dma_start(
                    x8, xtg[r0:r0 + P, hf * SQC:(hf + 1) * SQC])
                nc.vector.tensor_copy(
                    out=xt_sb[ci][:, ck * SQC:(ck + 1) * SQC], in_=x8)
        wvt_sb = load_w("wvts", 2, nc.sync)
        wot_sb = load_w("wots", 3, nc.sync)

        # bo: broadcast wire row [1, C] to [P, C] fp32 via a K=1 matmul
        bo_row = const.tile([1, C], BF16, name="bo_row", tag="bo_row")
        nc.scalar.dma_start(bo_row, bo_ap)
        ones_p = const.tile([1, P], BF16, name="ones_p", tag="ones_p")
        nc.vector.memset(ones_p, 1.0)
        bob_ps = psum.tile([P, C], F32, name="bob_ps", tag="proj")
        nc.tensor.matmul(bob_ps, lhsT=ones_p, rhs=bo_row,
                         start=True, stop=True)
        bob_sb = const.tile([P, C], F32, name="bobs", tag="bobs")
        nc.vector.tensor_copy(out=bob_sb, in_=bob_ps)

        ones_sb = const.tile([P, D], F16, name="ones_sb", tag="ones_sb")
        nc.vector.memset(ones_sb, 1.0)

        obounce = dram.tile([SQ + 8, C], I8, name="obounce", tag="obounce")
        og = dram.tile([NCORES * (SQ + 8), C], I8, name="og", tag="og")
        for rep in range(reps):
            emit_body(nc, tc, const, work, psum, row_pack,
                      xt_sb, xtq_sb, wqt_sb, wkt_sb, wvt_sb, wot_sb,
                      bob_sb, ones_sb, obounce)
        nc.gpsimd.collective_compute(
            "AllGather", mybir.AluOpType.bypass,
            replica_groups=[list(range(NCORES))],
            ins=[obounce.opt()], outs=[og.opt()])
        nc.gpsimd.dma_start(out_d, og[:])

    nc.compile()
    return nc


def emit_body(nc, tc, const, work, psum, row_pack,
              xt_sb, xtq_sb, wqt_sb, wkt_sb, wvt_sb, wot_sb,
              bob_sb, ones_sb, out_d):
    vp_sb = [None] * NSK

    def emit_vproj(t_i):
        vps = psum.tile([P, C], F32, name="vps", tag="proj")
        for ci in range(NCI):
            nc.tensor.matmul(vps, lhsT=xt_sb[ci][:, t_i * P:(t_i + 1) * P],
                             rhs=wvt_sb[ci],
                             start=(ci == 0), stop=(ci == NCI - 1))
        vp = const.tile([P, H * E], BF16, name=f"vp{t_i}", tag=f"vp{t_i}")
        vp3 = vp.rearrange("p (h e) -> p h e", e=E)
        nc.vector.tensor_copy(out=vp3[:, :, 0:D],
                              in_=vps.rearrange("p (h d) -> p h d", d=D))
        nc.vector.memset(vp3[:, :, D:E], 1.0)
        vp_sb[t_i] = vp

    def emit_qtp(p):
        qtp = work.tile([P, SQ], BF16, name="qtp", tag="qtp")
        for cq in range(NSQC):
            qps = psum.tile([P, SQC], F32, name="qps", tag="proj")
            for ci in range(NCI):
                nc.tensor.matmul(
                    qps, lhsT=wqt_sb[ci][:, p * P:(p + 1) * P],
                    rhs=xtq_sb[ci][:, cq * SQC:(cq + 1) * SQC],
                    start=(ci == 0), stop=(ci == NCI - 1))
            nc.vector.tensor_copy(out=qtp[:, cq * SQC:(cq + 1) * SQC], in_=qps)
        return qtp

    def emit_ktp_chunk(ktp, p, ck):
        kps = psum.tile([P, SQC], F32, name="kps", tag="proj")
        for ci in range(NCI):
            nc.tensor.matmul(
                kps, lhsT=wkt_sb[ci][:, p * P:(p + 1) * P],
                rhs=xt_sb[ci][:, ck * SQC:(ck + 1) * SQC],
                start=(ci == 0), stop=(ci == NCI - 1))
        nc.vector.tensor_copy(out=ktp[:, ck * SQC:(ck + 1) * SQC], in_=kps)

    # Ot: normalized attention output, head-concat layout [c_in, sq]
    ot_sb = [const.tile([P, SQ], BF16, name=f"ot{i}", tag=f"ot{i}")
             for i in range(NCI)]

    def make_norm_tail(h, oraw, r):
        """Broadcast-matmul + normalize for head h. Deferred into the next
        head's loop so the PE-stream bcast matmul never waits on the DVE
        recip (PE is in-order; an early bcast would bubble the pipeline)."""
        def tail():
            rbp = psum.tile([D, SQ], F32, name="rbp", tag="st")
            for cq in range(NSQC):
                sl = slice(cq * SQC, (cq + 1) * SQC)
                nc.tensor.matmul(rbp[:, sl], lhsT=ones_sb[D:D + 1, :],
                                 rhs=r[D:D + 1, sl], start=True, stop=True)
            rb = work.tile([D, SQ], F32, name="rb", tag="rb", bufs=2)
            nc.vector.tensor_copy(out=rb, in_=rbp)
            if h % 2 == 0:
                nc.vector.tensor_mul(out=ot_sb[h // 2][0:D, :],
                                     in0=oraw[0:D, :], in1=rb)
            else:
                # DVE lanes are partition-locked; move to the upper half by DMA
                otmp = work.tile([D, SQ], BF16, name="otmp", tag="otmp",
                                 bufs=2)
                nc.vector.tensor_mul(out=otmp, in0=oraw[0:D, :], in1=rb)
                nc.gpsimd.dma_start(ot_sb[h // 2][D:2 * D, :], otmp)
        return tail

    outacc = const.tile([P, S], F16, name="outacc", tag="outacc")

    def make_oproj_tail(pair):
        """Accumulate pair `pair`'s output-projection contribution into
        outacc (SBUF). Deferred so only the final pair's slice is in the
        kernel tail."""
        def tail():
            for sqt in range(SQ // P):
                ops = psum.tile([P, C], F32, name="ops", tag="proj")
                nc.tensor.matmul(ops,
                                 lhsT=ot_sb[pair][:, sqt * P:(sqt + 1) * P],
                                 rhs=wot_sb[pair], start=True, stop=True)
                osl = outacc[:, sqt * C:(sqt + 1) * C]
                if pair == 0:
                    nc.vector.tensor_add(osl, ops, bob_sb)
                else:
                    nc.vector.tensor_add(osl, osl, ops)
            if pair == NCI - 1:
                # per-token int8 quantization: row-absmax per 128-token
                # block, RTN convert on ScalarE, scales shipped as fp32
                # rows bitcast into the int8 output (flat idx = token)
                scmax = const.tile([P, SQ // P], F32, name="scmax",
                                   tag="scmax")
                for sqt in range(SQ // P):
                    nc.vector.tensor_reduce(
                        out=scmax[:, sqt:sqt + 1],
                        in_=outacc[:, sqt * C:(sqt + 1) * C],
                        axis=mybir.AxisListType.X, op=mybir.AluOpType.max,
                        apply_absolute_value=True)
                stmp = const.tile([P, SQ // P], F32, name="stmp", tag="stmp")
                nc.scalar.activation(
                    out=stmp, in_=scmax,
                    func=mybir.ActivationFunctionType.Copy,
                    scale=1.0 / 127.0, bias=1e-30)
                sinv = const.tile([P, SQ // P], F32, name="sinv", tag="sinv")
                nc.vector.reciprocal(sinv, stmp)
                oq = const.tile([P, S], I8, name="oq", tag="oq")
                for sqt in range(SQ // P):
                    blk = slice(sqt * C, (sqt + 1) * C)
                    nc.scalar.activation(
                        out=oq[:, blk], in_=outacc[:, blk],
                        func=mybir.ActivationFunctionType.Copy,
                        scale=sinv[:, sqt:sqt + 1])
                    nc.gpsimd.dma_start(out_d[sqt * P:(sqt + 1) * P, :],
                                        oq[:, blk])
                nc.gpsimd.dma_start(
                    out_d[SQ:SQ + 8, :].bitcast(F32).rearrange("a b -> b a"),
                    scmax)
        return tail

    ktp = qtp = None
    pending_norm = None
    pending_oproj = None
    next_pair = None          # (qtp, ktp, n_chunks_pre_emitted) for pair p+1
    pre_chunks = 0
    for h in range(H):
        p, half = h // 2, h % 2
        lo, hi = half * D, half * D + D          # head's rows in pair tiles
        olo, ohi = D - half * D, 2 * D - half * D  # opposite half rows

        if half == 0:
            if next_pair is not None:
                qtp, ktp, pre_chunks = next_pair
                next_pair = None
            else:
                qtp = emit_qtp(p)
                ktp = work.tile([P, S], BF16, name="ktp", tag="ktp")
                pre_chunks = 0
        # per-head swap copies: same rows duplicated into the other
        # partition half so both sq-chunks can use disjoint PE row groups
        if row_pack:
            dma_eng = nc.gpsimd
            qts = work.tile([P, SQ], BF16, name="qts", tag="qts")
            dma_eng.dma_start(qts[olo:ohi, :], qtp[lo:hi, :])
            kts = work.tile([P, S], BF16, name="kts", tag="kts")

        def emit_k_chunk(ck):
            if half == 0 and ck >= pre_chunks:
                emit_ktp_chunk(ktp, p, ck)
            if row_pack:
                dma_eng.dma_start(
                    kts[olo:ohi, ck * SQC:(ck + 1) * SQC],
                    ktp[lo:hi, ck * SQC:(ck + 1) * SQC])

        emit_k_chunk(0)
        oacc = psum.tile([E, SQ], F32, name="oacc", tag="oacc", bufs=1)
        for t_i in range(NSK):
            # prefetch the next K chunk one window early so the QK matmuls
            # never wait on the projection->evict->swap-DMA chain
            if t_i % 4 == 1 and t_i // 4 + 1 < S // SQC:
                emit_k_chunk(t_i // 4 + 1)
            if vp_sb[t_i] is None:
                emit_vproj(t_i)
            if t_i == 8 and pending_norm is not None:
                h_prev, tail = pending_norm
                tail()
                pending_norm = None
                if h_prev % 2 == 1:
                    pending_oproj = make_oproj_tail(h_prev // 2)
            if t_i == 16 and pending_oproj is not None:
                pending_oproj()
                pending_oproj = None
            # prefetch the next pair's Q/K projections late in the second
            # head of the current pair, so the pair boundary never stalls
            # ScalarE on the projection chain
            if t_i == 24 and half == 1 and h + 1 < H and next_pair is None:
                nq = emit_qtp(p + 1)
                nk = work.tile([P, S], BF16, name="ktp", tag="ktp")
                for ck0 in range(2):
                    emit_ktp_chunk(nk, p + 1, ck0)
                next_pair = (nq, nk, 2)

            st = psum.tile([P, SQ], F32, name="st", tag="st", bufs=2)
            ksl = slice(t_i * P, (t_i + 1) * P)
            if row_pack:
                nc.tensor.matmul(st[:, 0:SQC], lhsT=ktp[lo:hi, ksl],
                                 rhs=qtp[lo:hi, 0:SQC],
                                 start=True, stop=True,
                                 tile_position=(lo, 0))
                nc.tensor.matmul(st[:, SQC:SQ], lhsT=kts[olo:ohi, ksl],
                                 rhs=qts[olo:ohi, SQC:SQ],
                                 start=True, stop=True,
                                 tile_position=(olo, 0))
            else:
                for cq in range(NSQC):
                    nc.tensor.matmul(
                        st[:, cq * SQC:(cq + 1) * SQC],
                        lhsT=ktp[lo:hi, ksl],
                        rhs=qtp[lo:hi, cq * SQC:(cq + 1) * SQC],
                        start=True, stop=True)
            pt = work.tile([P, SQ], BF16, name="pt", tag="pt", bufs=3)
            nc.scalar.activation(out=pt, in_=st,
                                 func=mybir.ActivationFunctionType.Exp)
            for cq in range(NSQC):
                nc.tensor.matmul(
                    oacc[:, cq * SQC:(cq + 1) * SQC],
                    lhsT=vp_sb[t_i][:, h * E:(h + 1) * E],
                    rhs=pt[:, cq * SQC:(cq + 1) * SQC],
                    start=(t_i == 0), stop=(t_i == NSK - 1))

        # evict oacc to SBUF immediately so the PSUM slot frees for the next
        # head; the bcast+normalize runs deferred, off the critical path
        oraw = work.tile([E, SQ], F32, name="oraw", tag="oraw", bufs=2)
        nc.vector.tensor_copy(out=oraw, in_=oacc)
        r = work.tile([E, SQ], F16, name="r", tag="r", bufs=2)
        with nc.allow_low_precision("softmax denom recip; fp16 ~1e-4 rel"):
            nc.vector.reciprocal(r[D:E, :], oraw[D:E, :])
        pending_norm = (h, make_norm_tail(h, oraw, r))

    if pending_oproj is not None:      # pair 2, if heads ended before t==16
        pending_oproj()
    pending_norm[1]()                  # final head's normalization
    make_oproj_tail(NCI - 1)()         # final pair's projection + store


def _prep_globals(hidden_states, Wq, Wk, Wv, Wo, bo, force_w=False):
    """Build the global (concat over cores) wire arrays.

    Returns (xglobal int8 [8*C, SQ], wglobal bf16 [8*(2P+1), C] or None if
    the device-resident copy is current, wkey).
    """
    bf16 = ml_dtypes.bfloat16
    scale = np.float32(D) ** -0.5

    # int8 wire format for X: global absmax step, folded into Wq/Wk/Wv so
    # the device only does the (exact) int8->bf16 widening.  absmax scaling
    # means rint never exceeds +-127, so no clip pass is needed.
    hs = np.asarray(hidden_states, np.float32)
    step = np.float32(max(np.abs(hs).max() / 127.0, 1e-30))

    xg = _CACHE.get("xg")
    if xg is None:
        xg = _CACHE["xg"] = np.empty((NCORES * C, SQ), np.int8)
    xq = np.rint(hs * (np.float32(1.0) / step)).astype(np.int8)
    for c in range(NCORES):
        b, q0 = c // 4, (c % 4) * SQ
        xg[c * C:(c + 1) * C, :] = xq[b, q0:q0 + SQ, :].T

    import hashlib
    h = hashlib.blake2b(digest_size=16)
    for a in (Wq, Wk, Wv, Wo, bo):
        h.update(np.ascontiguousarray(a, np.float32).tobytes())
    h.update(step.tobytes())
    wkey = h.digest()

    wg = None
    if force_w or _CACHE.get("wkey") != wkey:
        # packed weights: rows 0-511 Wq^T*scale*step, 512-1023 Wk^T*step,
        # 1024-1535 Wv^T*step, 1536-2047 Wo^T; 1/8 row shard + bo per core.
        wcat = np.concatenate([
            np.asarray(Wq, np.float32).T * (scale * step),
            np.asarray(Wk, np.float32).T * step,
            np.asarray(Wv, np.float32).T * step,
            np.asarray(Wo, np.float32).T,
        ], axis=0).astype(bf16, order="C")
        bo_row = np.asarray(bo, np.float32).astype(bf16).reshape(1, C)
        wg = np.empty((NCORES * (2 * P + 1), C), bf16)
        for c in range(NCORES):
            r0 = c * (2 * P + 1)
            wg[r0:r0 + 2 * P] = wcat[c * 2 * P:(c + 1) * 2 * P]
            wg[r0 + 2 * P:r0 + 2 * P + 1] = bo_row
    return xg, wg, wkey


def _dequant_out(og):
    """[8*(SQ+8), C] int8 device output -> [B, S, C] fp32."""
    og = np.asarray(og).reshape(NCORES, SQ + 8, C)
    out = np.empty((B, S, C), np.float32)
    inv127 = np.float32(1.0 / 127.0)
    for c in range(NCORES):
        b, q0 = c // 4, (c % 4) * SQ
        blk = og[c]
        sc = np.frombuffer(blk[SQ:SQ + 8].tobytes(), np.float32)  # idx=token
        out[b, q0:q0 + SQ, :] = blk[:SQ].astype(np.float32) \
            * (sc[:, None] * inv127)
    return out


_CACHE = {}


def _get_nc():
    if "nc" not in _CACHE:
        _CACHE["nc"] = build_nc()
    return _CACHE["nc"]


def _get_exec():
    """Build (once) a cached jitted executable around the bass custom call.

    run_bass_kernel_spmd re-creates the jax.jit closure and re-uploads the
    donation zero-buffers on every call; with the axon tunnel at ~30 MB/s
    that dominates the wall clock.  Here the jit and the (never-donated,
    fully-overwritten) zero output operands live across calls.
    """
    if "exec" in _CACHE:
        return _CACHE["exec"]
    import jax.numpy  # noqa: F401  (jax initialized before first use)
    from jax.sharding import Mesh, PartitionSpec, NamedSharding
    from jax.experimental.shard_map import shard_map
    from concourse.bass2jax import (
        _bass_exec_p, install_neuronx_cc_hook, partition_id_tensor)

    nc = _get_nc()
    install_neuronx_cc_hook()
    partition_name = (nc.partition_id_tensor.name
                      if nc.partition_id_tensor else None)
    in_names, out_names, out_avals, zero_outs = [], [], [], []
    for alloc in nc.m.functions[0].allocations:
        if not isinstance(alloc, mybir.MemoryLocationSet):
            continue
        name = alloc.memorylocations[0].name
        if alloc.kind == "ExternalInput":
            if name != partition_name:
                in_names.append(name)
        elif alloc.kind == "ExternalOutput":
            shape = tuple(alloc.tensor_shape)
            dtype = mybir.dt.np(alloc.dtype)
            out_names.append(name)
            out_avals.append(jax.core.ShapedArray(shape, dtype))
            zero_outs.append(np.zeros(shape, dtype))
    n_params = len(in_names)
    in_names_all = list(in_names) + out_names
    if partition_name is not None:
        in_names_all.append(partition_name)

    def _body(*args):
        operands = list(args)
        if partition_name is not None:
            operands.append(partition_id_tensor())
        outs = _bass_exec_p.bind(
            *operands,
            out_avals=tuple(out_avals),
            in_names=tuple(in_names_all),
            out_names=tuple(out_names),
            lowering_input_output_aliases=(),
            sim_require_finite=True,
            sim_require_nnan=True,
            nc=nc,
        )
        return tuple(outs)

    devices = jax.devices()[:NCORES]
    mesh = Mesh(np.asarray(devices), ("core",))
    n_outs = len(out_avals)
    # outputs are gathered on-device across all 8 cores -> replicated specs,
    # so the host fetches the full result from one device in one RPC
    sharded = jax.jit(
        shard_map(_body, mesh=mesh,
                  in_specs=(PartitionSpec("core"),) * n_params
                  + (PartitionSpec(),) * n_outs,
                  out_specs=(PartitionSpec(),) * n_outs,
                  check_rep=False),
        keep_unused=True,
    )
    # zero output operands: created ON DEVICE (no tunnel transfer),
    # device-resident and reused every call (not donated)
    import jax.numpy as jnp
    mk_zeros = jax.jit(
        lambda: tuple(jnp.zeros(a.shape, a.dtype) for a in out_avals),
        out_shardings=NamedSharding(mesh, PartitionSpec()),
    )
    dev_zeros = list(mk_zeros())
    sh_core = NamedSharding(mesh, PartitionSpec("core"))
    _CACHE["exec"] = (sharded, in_names, out_names, out_avals, dev_zeros,
                      sh_core)
    return _CACHE["exec"]


def run(inputs, trace=False, **kwargs):
    """Run on hardware; returns (full_output [B,S,C] fp32, results)."""
    if trace:
        nc = _get_nc()
        xg, wg, _ = _prep_globals(**inputs, force_w=True)
        in_maps = []
        for c in range(NCORES):
            r0 = c * (2 * P + 1)
            in_maps.append({
                "xblob": np.ascontiguousarray(xg[c * C:(c + 1) * C, :]),
                "wblob": np.ascontiguousarray(wg[r0:r0 + 2 * P + 1, :]),
            })
        res = run_bass_kernel_spmd(nc, in_maps, list(range(NCORES)),
                                   trace=True, **kwargs)
        return _dequant_out(res.results[0]["out"]), res

    sharded, in_names, out_names, out_avals, dev_zeros, sh_core = _get_exec()
    xg, wg, wkey = _prep_globals(**inputs)
    if wg is not None or _CACHE.get("wkey") != wkey:
        _CACHE["wdev"] = jax.device_put(wg, sh_core)
        _CACHE["wkey"] = wkey
    args = {"xblob": xg, "wblob": _CACHE["wdev"]}
    out_arrs = sharded(*[args[n] for n in in_names], *dev_zeros)
    og = np.asarray(out_arrs[out_names.index("out")])
    return _dequant_out(og), None


def kernel(**inputs):
    try:
        out, _ = run(inputs)
    except Exception:
        # The axon fleet occasionally reports NRT_EXEC_UNIT_UNRECOVERABLE
        # once after a prior session's comm state; rebuild and retry once.
        _CACHE.clear()
        out, _ = run(inputs)
    return out
